# revision 1
# baseline (speedup 1.0000x reference)
"""BiGAT (2-omic projection + GATv2 conv + ELU) as a distributed Bass/Tile
kernel for 8 Trainium2 NeuronCores.

Strategy (graph/data parallel):
  - Nodes permuted so core c owns a balanced set of 50 dst blocks (slot
    assignment load-balances per-block edge counts, host-side).
  - Phase A: per-core projection from HOST-TRANSPOSED fp16 features (one
    contiguous slab DMA per 512-node group); h kept RESIDENT in SBUF;
    xl = h@Wl transposed to row layout and written to DRAM.
  - Phase B: ONE AllGather of the xl table (12.8 MB -> best bandwidth
    tier).  While it runs on the collective cores, the xr side
    (xr = h@Wr) is computed from the resident h and transposed straight
    into an SBUF table - xr never touches DRAM.
  - Phase C: one pass, per 128-dst-node block:
      * two hardware gathers of xl[src] rows (int16 idx sections: src
        owned by cores 0-3 / 4-7),
      * xr[dst] expanded per-edge via fp8 one-hot matmul on PE fused
        with an identity-add of xl -> s = xl[src]+xr[dst] in PSUM
        (4-tile chunks, one PSUM bank each),
      * lrelu(s) = Relu(0.8*s) + 0.2*s: two scalar-engine passes per
        chunk reading PSUM, DVE add,
      * att-dot: DVE multiply (2x mode) + pairwise step + tensor_reduce,
      * exp on scalar engine (shift folded into bias); broadcast-expand
        on gpsimd; messages ex*xl on DVE (2x),
      * scatter-add via fp8 one-hot matmuls into PSUM accumulating
        [node, H*C | denom]; softmax normalized at the epilogue
        (identical math to per-edge alpha).
  - One-hot matrices (both orientations) are host-precomputed fp8,
    streamed one DMA per block on the SP queue.
"""

import sys
import numpy as np

sys.path.insert(0, "/opt/trn_rl_repo")

P = 128
H, C = 4, 32
HC = H * C
NEG_SLOPE = 0.2
E_SHIFT = -6.1     # exp(e + shift): keeps ex, ex*xl in fp16 range.
                   # -1.6 extra vs the full-logit kernel: the dropped
                   # 0.2*att.xr[dst] term (max +1.79) is a per-dst softmax
                   # shift that cancels in alpha; the shift re-centers it.


def configure(cores=8, n1=25000, n2=25000, d1=2000, d2=500,
              n1pad=3200, n2pad=3200, ng=512):
    global CORES, N1, N2, D1, D2, N1PC, N2PC, N1PAD, N2PAD
    global NPC, NB, NTOT, NG, D1P, D2P, HALF
    CORES, N1, N2, D1, D2 = cores, n1, n2, d1, d2
    N1PC, N2PC = N1 // CORES, N2 // CORES
    N1PAD, N2PAD = n1pad, n2pad
    NPC = N1PAD + N2PAD
    NB = NPC // 128
    NTOT = CORES * NPC
    NG = ng
    D1P = ((d1 + 127) // 128) * 128
    D2P = ((d2 + 127) // 128) * 128
    HALF = NTOT // 2
    assert HALF <= 32768          # int16 gather indices per section


configure()


# ---------------------------------------------------------------------------
# host-side prep
# ---------------------------------------------------------------------------

def _wrap_idx(arr):
    L = arr.shape[0]
    w = arr.reshape(L // 16, 16).T.astype(np.int16)
    return np.tile(w, (8, 1))


def prep_edges(edge_index):
    """Balanced slots, per-core gather indices, fp8 one-hots, (F_LO, F_HI).

    lo section: src owned by cores 0..3 (gathered-table idx < HALF).
    hi section: src owned by cores 4..7.
    """
    import ml_dtypes
    global SLOT_OF, CORE_OF
    src, dst = edge_index[0].astype(np.int64), edge_index[1].astype(np.int64)

    allN = np.arange(N1 + N2)
    is1 = allN < N1
    CORE_OF = np.where(is1, allN // N1PC, (allN - N1) // N2PC)

    lo_src = CORE_OF[src] < CORES // 2
    lodeg = np.zeros(N1 + N2, np.int64)
    hideg = np.zeros(N1 + N2, np.int64)
    np.add.at(lodeg, dst[lo_src], 1)
    np.add.at(hideg, dst[~lo_src], 1)

    # balance dst nodes across blocks within each (core, omic) group
    SLOT_OF = np.zeros(N1 + N2, np.int64)
    for c in range(CORES):
        for base, lo_n, hi_n, nb0 in ((0, c * N1PC, (c + 1) * N1PC,
                                       N1PAD // 128),
                                      (N1PAD, N1 + c * N2PC,
                                       N1 + (c + 1) * N2PC, N2PAD // 128)):
            nodes = np.arange(lo_n, hi_n)
            lod = lodeg[nodes]
            hid = hideg[nodes]
            order = np.argsort(-(np.maximum(lod, hid) * 4096 + lod + hid),
                               kind="stable")
            cap = -(-len(nodes) // nb0)
            blo = np.zeros(nb0, np.int64)
            bhi = np.zeros(nb0, np.int64)
            fill = np.zeros(nb0, np.int64)
            for idx in order:
                l, h = lod[idx], hid[idx]
                score = np.maximum(blo + l, bhi + h) * 4096 + blo + bhi
                score[fill >= cap] = np.iinfo(np.int64).max
                b = int(np.argmin(score))
                SLOT_OF[nodes[idx]] = base + b * 128 + fill[b]
                blo[b] += l
                bhi[b] += h
                fill[b] += 1

    sc, ss = CORE_OF[src], SLOT_OF[src]
    dc, ds = CORE_OF[dst], SLOT_OF[dst]
    lo = lo_src.astype(np.int64)
    gidx = np.where(lo == 1, sc * NPC + ss, (sc - CORES // 2) * NPC + ss)
    blk = ds // 128

    order = np.lexsort((gidx, 1 - lo, blk, dc))
    gidx, dc, ds, blk, lo = (a[order] for a in (gidx, dc, ds, blk, lo))

    key = dc * NB + blk
    n_lo = np.zeros(CORES * NB, np.int64)
    n_hi = np.zeros(CORES * NB, np.int64)
    np.add.at(n_lo, key, lo)
    np.add.at(n_hi, key, 1 - lo)
    F_LO = int(np.max((n_lo + 127) // 128))
    F_HI = int(np.max((n_hi + 127) // 128))
    F = F_LO + F_HI

    gi_lo = np.zeros((CORES, NB, F_LO * 128), np.int16)
    gi_hi = np.zeros((CORES, NB, F_HI * 128), np.int16)
    dmod = np.full((CORES, NB, F, 128), 999, np.int32)

    bounds = np.searchsorted(key, np.arange(CORES * NB + 1))
    for k in range(CORES * NB):
        c, b = divmod(k, NB)
        a0, a1 = bounds[k], bounds[k + 1]
        nl = int(n_lo[k]); nh = int(n_hi[k])
        mod = (ds[a0:a1] % 128).astype(np.int32)
        gi_lo[c, b, :nl] = gidx[a0:a0 + nl]
        gi_hi[c, b, :nh] = gidx[a0 + nl:a1]
        dm = dmod[c, b].reshape(-1)
        dm[:nl] = mod[:nl]
        dm[F_LO * 128:F_LO * 128 + nh] = mod[nl:]

    j = np.arange(128, dtype=np.int32)
    out = []
    for c in range(CORES):
        glo = np.concatenate([_wrap_idx(gi_lo[c, b]) for b in range(NB)],
                             axis=1)
        ghi = np.concatenate([_wrap_idx(gi_hi[c, b]) for b in range(NB)],
                             axis=1)
        dmc = dmod[c]
        oh1 = (dmc[:, :, :, None] == j).astype(ml_dtypes.float8_e4m3)
        oh1 = oh1.transpose(2, 0, 1, 3).reshape(128, NB, F * 128)
        ohT = (dmc[:, :, None, :] == j[:, None]).astype(ml_dtypes.float8_e4m3)
        ohT = ohT.transpose(2, 0, 1, 3).reshape(128, NB, F * 128)
        ohc = np.concatenate([oh1, ohT], axis=2)
        out.append(dict(
            gilo=glo, gihi=ghi,
            ohc=np.ascontiguousarray(ohc.reshape(128, NB * 2 * F * 128))))
    return out, F_LO, F_HI


def _groups(npad):
    out = []
    g0 = 0
    while g0 < npad:
        out.append((g0, min(NG, npad - g0)))
        g0 += NG
    return out


def _slab(xT, npad):
    D = xT.shape[0]
    nch = D // 128
    xv = xT.reshape(nch, 128, npad)
    slabs = [np.ascontiguousarray(
                xv[:, :, g0:g0 + ng].transpose(1, 0, 2).reshape(128, nch * ng))
             for g0, ng in _groups(npad)]
    return np.concatenate(slabs, axis=1)


def prep_shards(x_mrna, x_mirna):
    shards = []
    for c in range(CORES):
        xm = np.zeros((D1P, N1PAD), np.float16)
        nodes1 = np.arange(c * N1PC, (c + 1) * N1PC)
        xm[:D1, SLOT_OF[nodes1]] = x_mrna[nodes1].T.astype(np.float16)
        xr_ = np.zeros((D2P, N2PAD), np.float16)
        nodes2 = np.arange(N1 + c * N2PC, N1 + (c + 1) * N2PC)
        xr_[:D2, SLOT_OF[nodes2] - N1PAD] = x_mirna[nodes2 - N1].T.astype(
            np.float16)
        shards.append((_slab(xm, N1PAD), _slab(xr_, N2PAD)))
    return shards


# ---------------------------------------------------------------------------
# program builder
# ---------------------------------------------------------------------------

def build_program(F_LO, F_HI):
    import concourse.bass as bass
    import concourse.mybir as mybir
    import concourse.tile as tile
    from concourse import bacc
    from concourse.masks import make_identity

    dt = mybir.dt
    f32 = dt.float32
    f16 = dt.float16
    f8 = dt.float8e4
    Alu = mybir.AluOpType
    Act = mybir.ActivationFunctionType
    F = F_LO + F_HI

    nc = bacc.Bacc("TRN2", target_bir_lowering=False, debug=False,
                   num_devices=CORES)

    xmT = nc.dram_tensor("xmT", [128, (D1P // 128) * N1PAD], f16,
                         kind="ExternalInput")
    xmiT = nc.dram_tensor("xmiT", [128, (D2P // 128) * N2PAD], f16,
                          kind="ExternalInput")
    wp1 = nc.dram_tensor("wp1", [D1P, P], f16, kind="ExternalInput")
    bp1 = nc.dram_tensor("bp1", [P, 1], f32, kind="ExternalInput")
    wp2 = nc.dram_tensor("wp2", [D2P, P], f16, kind="ExternalInput")
    bp2 = nc.dram_tensor("bp2", [P, 1], f32, kind="ExternalInput")
    wl = nc.dram_tensor("wl", [P, HC], f16, kind="ExternalInput")
    wr = nc.dram_tensor("wr", [P, HC], f16, kind="ExternalInput")
    attb = nc.dram_tensor("attb", [128, 128], f16, kind="ExternalInput")
    bgat = nc.dram_tensor("bgat", [128, 128], f16, kind="ExternalInput")
    gilo = nc.dram_tensor("gilo", [128, NB * F_LO * 8], dt.int16,
                          kind="ExternalInput")
    gihi = nc.dram_tensor("gihi", [128, NB * F_HI * 8], dt.int16,
                          kind="ExternalInput")
    ohc = nc.dram_tensor("ohc", [128, NB * 2 * F * 128], f8,
                         kind="ExternalInput")
    outp = nc.dram_tensor("outp", [NPC, HC], f16, kind="ExternalOutput")

    xl_loc = nc.dram_tensor("xl_loc", [NPC, HC], f16)
    xl_full = nc.dram_tensor("xl_full", [NTOT, HC], f16,
                             addr_space="Shared" if CORES > 4 else "Local")

    nch1 = D1P // 128
    nch2 = D2P // 128
    NCH = ((nch1, 0, N1PAD), (nch2, N1PAD, N2PAD))

    with tile.TileContext(nc, num_cores=CORES) as tc:
        with tc.tile_pool(name="const", bufs=1) as cst:

            identh = cst.tile([128, 128], f16)
            make_identity(nc, identh[:])

            wp1_sb = cst.tile([128, nch1 * 128], f16)
            nc.sync.dma_start(
                wp1_sb[:].rearrange("d (i j) -> d i j", j=128),
                wp1.ap().rearrange("(i d) j -> d i j", d=128))
            wp2_sb = cst.tile([128, nch2 * 128], f16)
            nc.sync.dma_start(
                wp2_sb[:].rearrange("d (i j) -> d i j", j=128),
                wp2.ap().rearrange("(i d) j -> d i j", d=128))
            bp1_sb = cst.tile([128, 1], f32)
            nc.sync.dma_start(bp1_sb[:], bp1.ap())
            bp2_sb = cst.tile([128, 1], f32)
            nc.sync.dma_start(bp2_sb[:], bp2.ap())
            wl_sb = cst.tile([128, HC], f16)
            nc.sync.dma_start(wl_sb[:], wl.ap())
            wr_sb = cst.tile([128, HC], f16)
            nc.sync.dma_start(wr_sb[:], wr.ap())
            attb_sb = cst.tile([128, 128], f16)
            nc.sync.dma_start(attb_sb[:], attb.ap())
            bgat_sb = cst.tile([128, 128], f16)
            nc.sync.dma_start(bgat_sb[:], bgat.ap())
            eshift_sb = cst.tile([128, 1], f32)
            nc.vector.memset(eshift_sb[:], E_SHIFT)
            gilo_sb = cst.tile([128, NB * F_LO * 8], dt.int16)
            nc.sync.dma_start(gilo_sb[:], gilo.ap())
            gihi_sb = cst.tile([128, NB * F_HI * 8], dt.int16)
            nc.sync.dma_start(gihi_sb[:], gihi.ap())

            hT_all = cst.tile([128, NPC], f16)     # resident projections
            xrk_all = cst.tile([128, NPC], f16)    # resident xr row table

            # ---------------- phase A: projections + xl table --------------
            pa_ctx = tc.tile_pool(name="pa", bufs=3)
            pa = pa_ctx.__enter__()
            pa_ps_ctx = tc.tile_pool(name="pa_ps", bufs=2, space="PSUM")
            pa_ps = pa_ps_ctx.__enter__()
            pa_tps_ctx = tc.tile_pool(name="pa_tps", bufs=4, space="PSUM")
            pa_tps = pa_tps_ctx.__enter__()
            for sec, (nchs, row0, npad) in enumerate(NCH):
                xdram = (xmT, xmiT)[sec]
                wp_sb = (wp1_sb, wp2_sb)[sec]
                bp_sb = (bp1_sb, bp2_sb)[sec]
                dq = (nc.scalar, nc.sync)[sec]
                goff = 0
                for g0, ng in _groups(npad):
                    nt = ng // 128
                    hps = pa_ps.tile([128, NG], f32, tag="hps")
                    xTg = pa.tile([128, nchs * NG], f16, tag="xTg")
                    dq.dma_start(
                        xTg[:, :nchs * ng],
                        xdram.ap()[:, goff:goff + nchs * ng])
                    goff += nchs * ng
                    for i in range(nchs):
                        nc.tensor.matmul(
                            hps[:, :ng],
                            lhsT=wp_sb[:, i * 128:(i + 1) * 128],
                            rhs=xTg[:, i * ng:(i + 1) * ng],
                            start=(i == 0), stop=(i == nchs - 1))
                    r0 = row0 + g0
                    nc.vector.tensor_scalar(hT_all[:, r0:r0 + ng],
                                            hps[:, :ng],
                                            bp_sb[:, 0:1], None, op0=Alu.add)
                    xps = pa_ps.tile([128, NG], f32, tag="xps")
                    nc.tensor.matmul(xps[:, :ng], lhsT=wl_sb[:],
                                     rhs=hT_all[:, r0:r0 + ng],
                                     start=True, stop=True)
                    xsb = pa.tile([128, NG], f16, tag="xsb")
                    nc.vector.tensor_copy(xsb[:, :ng], xps[:, :ng])
                    rsb = pa.tile([128, NG], f16, tag="rsb")
                    for ti in range(nt):
                        tp = pa_tps.tile([128, 128], f16, tag="tp")
                        nc.tensor.transpose(
                            tp[:], xsb[:, ti * 128:(ti + 1) * 128], identh[:])
                        nc.vector.tensor_copy(
                            rsb[:, ti * 128:(ti + 1) * 128], tp[:])
                    dq.dma_start(
                        xl_loc.ap()[r0:r0 + ng, :]
                        .rearrange("(t p) j -> p t j", p=128),
                        rsb[:, :ng].rearrange("p (t j) -> p t j", j=128))

            pa_tps_ctx.__exit__(None, None, None)
            pa_ps_ctx.__exit__(None, None, None)
            pa_ctx.__exit__(None, None, None)

            # ---------------- phase B: single AllGather ---------------------
            nc.gpsimd.collective_compute(
                "AllGather", Alu.bypass,
                ins=[xl_loc.ap()],
                outs=[xl_full.ap()],
                replica_groups=[list(range(CORES))])

            # ---- xr side from resident h: runs during the AllGather -------
            with tc.tile_pool(name="px", bufs=3) as px, \
                 tc.tile_pool(name="px_ps", bufs=2, space="PSUM") as px_ps, \
                 tc.tile_pool(name="px_tps", bufs=4, space="PSUM") as px_tps:
                for g0, ng in _groups(NPC):
                    nt = ng // 128
                    xps = px_ps.tile([128, NG], f32, tag="xps2")
                    nc.tensor.matmul(xps[:, :ng], lhsT=wr_sb[:],
                                     rhs=hT_all[:, g0:g0 + ng],
                                     start=True, stop=True)
                    xsb = px.tile([128, NG], f16, tag="xsb2")
                    nc.vector.tensor_copy(xsb[:, :ng], xps[:, :ng])
                    for ti in range(nt):
                        tp = px_tps.tile([128, 128], f16, tag="tp2")
                        nc.tensor.transpose(
                            tp[:], xsb[:, ti * 128:(ti + 1) * 128], identh[:])
                        nc.vector.tensor_copy(
                            xrk_all[:, g0 + ti * 128:g0 + (ti + 1) * 128],
                            tp[:])

            # ---------------- phase C: edge processing ---------------------
            NCK = 8                                   # sps tiles per chunk
            with tc.tile_pool(name="stgp", bufs=1) as stgp, \
                 tc.tile_pool(name="pc", bufs=2) as pc, \
                 tc.tile_pool(name="pc_sps", bufs=3, space="PSUM") as pc_sps, \
                 tc.tile_pool(name="pc_acc", bufs=2, space="PSUM") as pc_acc:
                stg = stgp.tile([128, NB * 132], f16, tag="stg")
                for b in range(NB):
                    xlb_t = pc.tile([128, F * 128], f16, tag="xlb", bufs=5)
                    xlb = xlb_t[:]
                    xlb3 = xlb.rearrange("p (f x) -> p f x", f=F)
                    nc.gpsimd.dma_gather(
                        out_ap=xlb3[:, 0:F_LO, :],
                        in_ap=xl_full.ap()[0:HALF, :],
                        idxs_ap=gilo_sb[:, b * F_LO * 8:(b + 1) * F_LO * 8],
                        num_idxs=F_LO * 128, num_idxs_reg=F_LO * 128,
                        elem_size=HC, single_packet=False)
                    nc.gpsimd.dma_gather(
                        out_ap=xlb3[:, F_LO:F, :],
                        in_ap=xl_full.ap()[HALF:NTOT, :],
                        idxs_ap=gihi_sb[:, b * F_HI * 8:(b + 1) * F_HI * 8],
                        num_idxs=F_HI * 128, num_idxs_reg=F_HI * 128,
                        elem_size=HC, single_packet=False)
                    ohc_t_t = pc.tile([128, 2 * F * 128], f8, tag="ohct",
                                      bufs=6)
                    ohc_t = ohc_t_t[:]
                    nc.sync.dma_start(
                        ohc_t,
                        ohc.ap()[:, b * 2 * F * 128:(b + 1) * 2 * F * 128])
                    oh1_t = ohc_t[:, 0:F * 128]
                    ohT_t = ohc_t[:, F * 128:2 * F * 128]

                    # Tables hold 0.2*xl / 0.2*xr (0.2 folded into Wl/Wr;
                    # epilogue multiplies the softmax reciprocal by 5), so
                    # sps = 0.2*s.  The logit att.lrelu(s) splits as
                    #   att.Relu(0.8 s) + 0.2 att.xl[src] + 0.2 att.xr[dst]
                    # and the last term is a per-dst softmax shift that
                    # cancels in alpha -> dropped (E_SHIFT re-centers it).
                    # Relu(0.8 s) = Relu(4*sps): one Act pass per chunk;
                    # u = r08 + 0.2 xl[src] = r08 + xlb: one DVE add.
                    r08 = pc.tile([128, F * 128], f16, tag="r08")
                    for h0 in range(0, F, NCK):
                        h1 = min(h0 + NCK, F)
                        nh = h1 - h0
                        sps = pc_sps.tile([128, NCK * 128], f32, tag="sps")
                        for t in range(h0, h1):
                            o = (t - h0) * 128
                            nc.tensor.matmul(
                                sps[:, o:o + 128],
                                lhsT=ohT_t[:, t * 128:(t + 1) * 128],
                                rhs=xrk_all[:, b * 128:(b + 1) * 128],
                                start=True, stop=False)
                            nc.tensor.matmul(
                                sps[:, o:o + 128],
                                lhsT=identh[:],
                                rhs=xlb[:, t * 128:(t + 1) * 128],
                                start=False, stop=True)
                        nc.scalar.activation(r08[:, h0 * 128:h1 * 128],
                                             sps[:, :nh * 128], Act.Relu,
                                             scale=(1.0 - NEG_SLOPE) /
                                                   NEG_SLOPE)
                    g = pc.tile([128, F * 128], f16, tag="g")
                    nc.vector.tensor_add(g[:], r08[:], xlb)

                    pm = pc.tile([128, F * 128], f16, tag="pm")
                    nc.vector.tensor_tensor(
                        pm[:].rearrange("p (f x) -> p f x", f=F),
                        g[:].rearrange("p (f x) -> p f x", f=F),
                        attb_sb[:].unsqueeze(1).broadcast_to([128, F, 128]),
                        op=Alu.mult)
                    tr1 = pc.tile([128, F * 64], f16, tag="tr1")
                    pm4 = pm[:].rearrange("p (f h x) -> p f h x", f=F, h=H)
                    t1 = tr1[:].rearrange("p (f h x) -> p f h x", f=F, h=H)
                    nc.gpsimd.tensor_tensor(t1, pm4[:, :, :, 0:16],
                                            pm4[:, :, :, 16:32], op=Alu.add)
                    tr2 = pc.tile([128, F * 32], f16, tag="tr2")
                    t2 = tr2[:].rearrange("p (f h x) -> p f h x", f=F, h=H)
                    nc.gpsimd.tensor_tensor(t2, t1[:, :, :, 0:8],
                                            t1[:, :, :, 8:16], op=Alu.add)
                    e4f = pc.tile([128, F * 4], f32, tag="e4f")
                    nc.vector.tensor_reduce(
                        e4f[:].rearrange("p (f h) -> p f h", f=F),
                        t2, axis=mybir.AxisListType.X, op=Alu.add)
                    msgex = pc.tile([128, F * 132], f16, tag="msgex")
                    mv = msgex[:].rearrange("p (f x) -> p f x", f=F, x=132)
                    nc.scalar.activation(
                        mv[:, :, 128:132],
                        e4f[:].rearrange("p (f h) -> p f h", f=F), Act.Exp,
                        bias=eshift_sb[:, 0:1])
                    exe = pc.tile([128, F * 128], f16, tag="exe")
                    ex4 = exe[:].rearrange("p (f h x) -> p f h x", f=F, h=H)
                    FH = F // 2
                    nc.gpsimd.tensor_copy(
                        ex4[:, 0:FH], mv[:, 0:FH, 128:132].unsqueeze(3)
                        .broadcast_to([128, FH, H, C]))
                    nc.scalar.copy(
                        ex4[:, FH:F], mv[:, FH:F, 128:132].unsqueeze(3)
                        .broadcast_to([128, F - FH, H, C]))
                    nc.vector.tensor_tensor(
                        mv[:, :, 0:128], xlb3,
                        exe[:].rearrange("p (f x) -> p f x", f=F),
                        op=Alu.mult)
                    acc = pc_acc.tile([128, 132], f32, tag="acc")
                    for t in range(F):
                        nc.tensor.matmul(
                            acc[:],
                            lhsT=oh1_t[:, t * 128:(t + 1) * 128],
                            rhs=mv[:, t, :],
                            start=(t == 0), stop=(t == F - 1))
                    nc.scalar.copy(stg[:, b * 132:(b + 1) * 132], acc[:])

                # ---- batched epilogue (split across DVE and gpsimd) ----
                sv = stg[:].rearrange("p (b x) -> p b x", b=NB)
                dn = stgp.tile([128, NB * 4], f32, tag="dn")
                nc.vector.tensor_scalar(
                    dn[:], sv[:, :, 128:132], 1e-16, None, op0=Alu.add)
                rc = stgp.tile([128, NB * 4], f32, tag="rc")
                nc.vector.reciprocal(rc[:], dn[:])
                # undo the 0.2 table scaling on the message numerators
                nc.vector.tensor_scalar(rc[:], rc[:], 1.0 / NEG_SLOPE, None,
                                        op0=Alu.mult)
                o1 = stgp.tile([128, NB * 128], f16, tag="o1")
                o2 = stgp.tile([128, NB * 128], f16, tag="o2")
                NBH = NB // 2
                for eng, b0, b1 in ((nc.vector, 0, NBH),
                                    (nc.gpsimd, NBH, NB)):
                    nb = b1 - b0
                    o1v = o1[:, b0 * 128:b1 * 128]
                    eng.tensor_tensor(
                        o1v.rearrange("p (b h c) -> p b h c", b=nb, h=H),
                        sv[:, b0:b1, 0:128]
                        .rearrange("p b (h c) -> p b h c", h=H),
                        rc[:, b0 * 4:b1 * 4]
                        .rearrange("p (b h) -> p b h", b=nb)
                        .unsqueeze(3).broadcast_to([128, nb, H, C]),
                        op=Alu.mult)
                    eng.tensor_tensor(
                        o1v.rearrange("p (b x) -> p b x", b=nb),
                        o1v.rearrange("p (b x) -> p b x", b=nb),
                        bgat_sb[:].unsqueeze(1).broadcast_to([128, nb, 128]),
                        op=Alu.add)
                    o2v = o2[:, b0 * 128:b1 * 128]
                    eng.tensor_scalar(o2v, o1v, -1.0, 1.0,
                                      op0=Alu.min, op1=Alu.add)
                    nc.scalar.activation(o2v, o2v, Act.Exp)
                    eng.tensor_scalar(o1v, o1v, -1.0, None, op0=Alu.max)
                    eng.tensor_add(o1v, o1v, o2v)
                    nc.sync.dma_start(
                        outp.ap()[b0 * 128:b1 * 128, :]
                        .rearrange("(b p) j -> p b j", p=128),
                        o1v.rearrange("p (b j) -> p b j", b=nb))

    nc.compile()
    return nc


# ---------------------------------------------------------------------------
# entry point
# ---------------------------------------------------------------------------

def _make_in_maps(inputs):
    x_mrna = np.asarray(inputs["x_mrna"], np.float32)
    x_mirna = np.asarray(inputs["x_mirna"], np.float32)
    att = np.asarray(inputs["att"], np.float32)
    edge_index = np.asarray(inputs["edge_index"])

    edge_arrays, F_LO, F_HI = prep_edges(edge_index)
    shards = prep_shards(x_mrna, x_mirna)

    att_flat = att.reshape(HC)
    attb = np.tile(att_flat[None, :], (128, 1)).astype(np.float16)
    bgatb = np.tile(np.asarray(inputs["b_gat"], np.float32)[None, :] - 1.0,
                    (128, 1)).astype(np.float16)

    wp1p = np.zeros((D1P, P), np.float16)
    wp1p[:D1] = np.asarray(inputs["Wp1"], np.float32).astype(np.float16)
    wp2p = np.zeros((D2P, P), np.float16)
    wp2p[:D2] = np.asarray(inputs["Wp2"], np.float32).astype(np.float16)
    common = dict(
        wp1=wp1p,
        bp1=np.asarray(inputs["bp1"], np.float32).reshape(P, 1),
        wp2=wp2p,
        bp2=np.asarray(inputs["bp2"], np.float32).reshape(P, 1),
        wl=(NEG_SLOPE * np.asarray(inputs["Wl"], np.float32))
        .astype(np.float16),
        wr=(NEG_SLOPE * np.asarray(inputs["Wr"], np.float32))
        .astype(np.float16),
        attb=attb, bgat=bgatb)

    in_maps = []
    for c in range(CORES):
        xmc, xrc = shards[c]
        m = dict(common)
        m.update(xmT=xmc, xmiT=xrc, **edge_arrays[c])
        in_maps.append(m)
    return in_maps, F_LO, F_HI


def _assemble(results):
    out = np.empty((N1 + N2, HC), np.float32)
    for c in range(CORES):
        o = np.asarray(results[c]["outp"], np.float32)
        nodes1 = np.arange(c * N1PC, (c + 1) * N1PC)
        out[nodes1] = o[SLOT_OF[nodes1]]
        nodes2 = np.arange(N1 + c * N2PC, N1 + (c + 1) * N2PC)
        out[nodes2] = o[SLOT_OF[nodes2]]
    return out


def kernel(**inputs):
    from concourse.bass_utils import run_bass_kernel_spmd

    in_maps, F_LO, F_HI = _make_in_maps(inputs)
    nc = build_program(F_LO, F_HI)
    res = run_bass_kernel_spmd(nc, in_maps, list(range(CORES)))
    return _assemble(res.results)


if __name__ == "__main__":
    rng = np.random.default_rng(0)
    ei = rng.integers(0, N1 + N2, size=(2, 800000), dtype=np.int32)
    arrs, flo, fhi = prep_edges(ei)
    print("F_LO", flo, "F_HI", fhi)

